# revision 1
# baseline (speedup 1.0000x reference)
"""Trainium2 Bass kernel for nn_DGLayer_16286515986763.

Math (reference unrolled, N_STEPS=5, FFI_DELAY=2, FBI_DELAY=20 > N_STEPS so
the FBI masks are dead code):

    drive = amp * clip(ffi_scale,0.01) * 0.5 * (1 + cos(phase))
    md    = mean(drive);  m0 = 0.3*md;  m1 = 0.51*md
    p0    = relu(drive - m0)
    m2    = 0.357*md + 0.3*mean(p0)
    ema5  = 0.17493*drive + 0.147*p0 + 0.21*relu(drive-m1) + 0.3*relu(drive-m2)
    out   = where(ema5 >= kth_largest(ema5, 32), ema5, 0)

Key facts used:
  * ema5 is a strictly increasing per-row function of drive, so the top-32
    mask of ema5 equals the top-32 mask of drive (dd below).
  * The top-32 threshold is far above m0/m1/m2 (checked per-row via stats;
    host-fixed otherwise), so on selected elements every relu is affine:
    ema5 = A*dd + B_row with per-row B from two row-sums.
  * Top-32 per row: per-chunk top-8 via the DVE Max8 op, then 4 rounds of
    max+match_replace over the candidates. Exact unless a chunk holds >=9 of
    the row's top-32; detected per row (m8 >= th) and recomputed on host.

Sharding: pure data parallel, 4096 rows per core on 8 cores.
"""
import sys

for _p in ("/opt/trn_rl_repo", "/root/.axon_site/_ro/trn_rl_repo"):
    if _p not in sys.path:
        sys.path.insert(0, _p)

import numpy as np

import concourse.bass as bass
import concourse.bacc as bacc
import concourse.tile as tile
import concourse.mybir as mybir
from concourse.bass_utils import run_bass_kernel_spmd

AF = mybir.ActivationFunctionType
OP = mybir.AluOpType
F32 = mybir.dt.float32

B_FULL, N = 32768, 1024
NCORES = 8
ROWS = B_FULL // NCORES      # 4096 rows per core
P = 128                      # SBUF partitions
TILES = ROWS // P            # 32 tiles per core
NEG_INF = -3.0e38
HALF_PI = float(np.float32(np.pi / 2))

# default configuration (engine assignment + selection chunk width)
CFG = dict(
    chunk=64,          # selection chunk width (32 -> 32 max calls, 64 -> 16)
    dd_engine="dve",   # "pool" (TT mult) or "dve" (STT with accum)
    sdd_mode="stt",    # "act" copy+accum pass, or "stt" (requires dd_engine=dve)
    z_engine="dve",    # "act" relu(A*dd+B) or "dve" tensor_scalar
    out_engine="pool", # "pool" mask*z TT mult or "dve" STT
    mask_engine="dve", # "dve" tensor_scalar or "pool" broadcast TT is_ge
    tiny_engine="dve", # "dve" or "act" for (128,1) scalar ops
    io_bufs=4, mid_bufs=3, sel_bufs=3,
    repeats=1,         # python-unrolled repeats of the whole pipeline
    loop_repeats=1,    # hardware For_i loop around the pipeline (timing)
)

_cache = {}


def _build(s: float, cfg: dict | None = None):
    cfg = {**CFG, **(cfg or {})}
    key = (s, tuple(sorted(cfg.items())))
    if key in _cache:
        return _cache[key]

    C = cfg["chunk"]
    G = N // C
    NCAND = G * 8

    A_imm = float(np.float32(s * 0.83193))
    c_beta0 = float(np.float32(-0.3 / N))
    c_B1 = float(np.float32(-s * 0.25836 / N))
    c_B2 = float(np.float32(-s * 0.09 / N))

    nc = bacc.Bacc("TRN2", target_bir_lowering=False, debug=False)

    _pihalf = nc.alloc_sbuf_tensor("const-pihalf", [P, 1], F32)
    nc.gpsimd.memset(_pihalf.ap(), HALF_PI)
    nc.const_aps.aps[(F32, HALF_PI)] = _pihalf.ap()
    nc.all_engine_barrier()

    phase_d = nc.dram_tensor("phase", [ROWS, N], F32, kind="ExternalInput")
    amp_d = nc.dram_tensor("amp", [ROWS, N], F32, kind="ExternalInput")
    out_d = nc.dram_tensor("out", [ROWS, N], F32, kind="ExternalOutput")
    # per-row stats: [unused, m8, Sdd, S0]; th goes out separately from r4
    stats_d = nc.dram_tensor("stats", [ROWS, 4], F32, kind="ExternalOutput")
    th_d = nc.dram_tensor("th", [ROWS, 1], F32, kind="ExternalOutput")

    phase_t = phase_d.ap().rearrange("(t p) n -> t p n", p=P)
    amp_t = amp_d.ap().rearrange("(t p) n -> t p n", p=P)
    out_t = out_d.ap().rearrange("(t p) n -> t p n", p=P)
    stats_t = stats_d.ap().rearrange("(t p) n -> t p n", p=P)
    th_t = th_d.ap().rearrange("(t p) n -> t p n", p=P)

    tiny = nc.vector if cfg["tiny_engine"] == "dve" else nc.gpsimd

    iob = cfg.get("io_bufs", 3)
    midb = cfg.get("mid_bufs", 2)
    selb = cfg.get("sel_bufs", 2)
    with tile.TileContext(nc) as tc:
        import contextlib
        lr = cfg.get("loop_repeats", 1)
        with tc.tile_pool(name="io", bufs=iob) as io, \
             tc.tile_pool(name="mid", bufs=midb) as mid, \
             tc.tile_pool(name="sel", bufs=selb) as selp, \
             (tc.For_i(0, lr, 1, staggered_reset=True,
                       hint_engines=(mybir.EngineType.DVE, mybir.EngineType.Activation,
                                     mybir.EngineType.Pool, mybir.EngineType.SP))
              if lr > 1 else contextlib.nullcontext()):
            for rep in range(cfg["repeats"]):
                for t in range(TILES):
                    phase = io.tile([P, N], F32, tag="phase")
                    nc.sync.dma_start(phase[:], phase_t[t])
                    amp = io.tile([P, N], F32, tag="amp")
                    nc.sync.dma_start(amp[:], amp_t[t])

                    stats = selp.tile([P, 4], F32, tag="stats")

                    # h = cos(phase/2); g = h^2 = (1+cos(phase))/2
                    h = mid.tile([P, N], F32, tag="h")
                    nc.scalar.activation(h[:], phase[:], AF.Sin,
                                         bias=HALF_PI, scale=-0.5)
                    g = mid.tile([P, N], F32, tag="g")
                    nc.scalar.activation(g[:], h[:], AF.Square)

                    # dd = g * amp ; Sdd (row sum) -> stats[:,2]
                    dd = mid.tile([P, N], F32, tag="dd")
                    if cfg["dd_engine"] == "dve":
                        nc.vector.scalar_tensor_tensor(
                            dd[:], g[:], 0.0, amp[:], OP.add, OP.mult,
                            accum_out=stats[:, 2:3] if cfg["sdd_mode"] == "stt"
                            else None)
                    else:
                        nc.gpsimd.tensor_tensor(dd[:], g[:], amp[:], OP.mult)
                    if cfg["sdd_mode"] == "act" or cfg["dd_engine"] == "pool":
                        sddscr = mid.tile([P, N], F32, tag="sddscr")
                        nc.scalar.activation(sddscr[:], dd[:], AF.Copy,
                                             accum_out=stats[:, 2:3])

                    # beta0 = Sdd * (-0.3/N)
                    beta0 = selp.tile([P, 1], F32, tag="beta0")
                    if cfg["tiny_engine"] == "act":
                        nc.scalar.activation(beta0[:], stats[:, 2:3],
                                             AF.Copy, scale=c_beta0)
                    else:
                        tiny.tensor_scalar(beta0[:], stats[:, 2:3], c_beta0,
                                           None, OP.mult)

                    # relu(dd + beta0): only its accumulator S0 is needed
                    q = mid.tile([P, N], F32, tag="q")
                    nc.scalar.activation(q[:], dd[:], AF.Relu, bias=beta0[:],
                                         scale=1.0, accum_out=stats[:, 3:4])

                    # B_row = Sdd*c_B1 + S0*c_B2
                    v2 = selp.tile([P, 1], F32, tag="v2")
                    Bv = selp.tile([P, 1], F32, tag="Bv")
                    if cfg["tiny_engine"] == "act":
                        nc.scalar.activation(v2[:], stats[:, 3:4],
                                             AF.Copy, scale=c_B2)
                        nc.scalar.activation(Bv[:], stats[:, 2:3],
                                             AF.Identity, bias=v2[:],
                                             scale=c_B1)
                    else:
                        tiny.tensor_scalar(v2[:], stats[:, 3:4], c_B2, None,
                                           OP.mult)
                        nc.vector.scalar_tensor_tensor(
                            Bv[:], stats[:, 2:3], c_B1, v2[:], OP.mult, OP.add)

                    # --- selection on dd ---
                    cand = selp.tile([P, NCAND], F32, tag="cand")
                    for j in range(G):
                        nc.vector.max(cand[:, j * 8:(j + 1) * 8],
                                      dd[:, j * C:(j + 1) * C])
                    mrA = selp.tile([P, NCAND], F32, tag="mrA")
                    mrB = selp.tile([P, NCAND], F32, tag="mrB")
                    r1 = selp.tile([P, 8], F32, tag="r1")
                    r2 = selp.tile([P, 8], F32, tag="r2")
                    r3 = selp.tile([P, 8], F32, tag="r3")
                    r4 = selp.tile([P, 8], F32, tag="r4")
                    nc.vector.max(r1[:], cand[:])
                    nc.vector.match_replace(mrA[:], r1[:], cand[:], NEG_INF)
                    nc.vector.max(r2[:], mrA[:])
                    nc.vector.match_replace(mrB[:], r2[:], mrA[:], NEG_INF)
                    nc.vector.max(r3[:], mrB[:])
                    nc.vector.match_replace(mrA[:], r3[:], mrB[:], NEG_INF)
                    nc.vector.max(r4[:], mrA[:])
                    nc.vector.tensor_reduce(stats[:, 1:2], cand[:, 7::8],
                                            mybir.AxisListType.X, OP.max)
                    nc.sync.dma_start(th_t[t], r4[:, 7:8])

                    # z = A*dd + B
                    z = mid.tile([P, N], F32, tag="z")
                    if cfg["z_engine"] == "act":
                        # relu ok: z only read where mask==1, where z>0
                        nc.scalar.activation(z[:], dd[:], AF.Relu,
                                             bias=Bv[:], scale=A_imm)
                    else:
                        nc.vector.tensor_scalar(z[:], dd[:], A_imm, Bv[:],
                                                OP.mult, OP.add)

                    # out = (dd >= th) * z
                    out = mid.tile([P, N], F32, tag="out")
                    if cfg["out_engine"] == "pool":
                        mask = mid.tile([P, N], F32, tag="mask")
                        if cfg["mask_engine"] == "pool":
                            th_b = r4[:, 7:8].to_broadcast((P, N))
                            nc.gpsimd.tensor_tensor(mask[:], dd[:], th_b,
                                                    OP.is_ge)
                        else:
                            nc.vector.tensor_scalar(mask[:], dd[:],
                                                    r4[:, 7:8], None, OP.is_ge)
                        nc.gpsimd.tensor_tensor(out[:], mask[:], z[:], OP.mult)
                    else:
                        nc.vector.scalar_tensor_tensor(
                            out[:], dd[:], stats[:, 0:1], z[:],
                            OP.is_ge, OP.mult)

                    nc.sync.dma_start(out_t[t], out[:])
                    nc.sync.dma_start(stats_t[t], stats[:])

    nc.compile()
    _cache[key] = nc
    return nc


def _reference_rows(phase, amp, s):
    """Exact f32 recompute of the reference for a few rows (host fixup)."""
    f32 = np.float32
    drive = (amp * f32(s) * f32(0.5) *
             (f32(1.0) + np.cos(phase, dtype=f32))).astype(f32)
    ema = np.zeros_like(drive)
    ffi_hist = []
    for t in range(5):
        ffi = ffi_hist[t - 2] if t >= 2 else np.zeros((drive.shape[0], 1), f32)
        inp = np.maximum(drive - ffi, 0)
        ema = (f32(0.7) * ema + f32(0.3) * inp).astype(f32)
        ffi_hist.append(ema.mean(1, keepdims=True, dtype=f32).astype(f32))
    kth = np.sort(ema, 1)[:, ::-1][:, 31:32]
    return np.where(ema >= kth, ema, 0).astype(f32)


def kernel(phase, amplitude, ffi_scale, fbi_temperature):
    phase = np.asarray(phase, dtype=np.float32)
    amplitude = np.asarray(amplitude, dtype=np.float32)
    s = float(np.clip(np.float32(ffi_scale), np.float32(0.01), None))

    nc = _build(s)
    in_maps = [
        {"phase": np.ascontiguousarray(phase[i * ROWS:(i + 1) * ROWS]),
         "amp": np.ascontiguousarray(amplitude[i * ROWS:(i + 1) * ROWS])}
        for i in range(NCORES)
    ]
    res = run_bass_kernel_spmd(nc, in_maps, list(range(NCORES)))
    out = np.concatenate([res.results[i]["out"] for i in range(NCORES)], axis=0)
    stats = np.concatenate([res.results[i]["stats"] for i in range(NCORES)],
                           axis=0)
    th = np.concatenate([res.results[i]["th"] for i in range(NCORES)],
                        axis=0)[:, 0]

    # Host-side validity flags (exactness guards); recompute flagged rows.
    m8, Sdd, S0 = stats[:, 1], stats[:, 2], stats[:, 3]
    mdd = Sdd / np.float32(N)
    mq0 = S0 / np.float32(N)
    m2 = np.float32(0.357) * mdd + np.float32(0.3) * mq0
    mmax = np.maximum(np.float32(0.51) * mdd, m2)
    bad = (m8 >= th) | (th <= np.float32(1.05) * mmax)
    import os
    if os.environ.get("DG_DEBUG"):
        print(f"[kernel] flagged rows: {int(bad.sum())}")
    if bad.any():
        idx = np.where(bad)[0]
        out[idx] = _reference_rows(phase[idx], amplitude[idx], s)
    return out



# revision 8
# speedup vs baseline: 1.0795x; 1.0795x over previous
"""Trainium2 Bass kernel for nn_DGLayer_16286515986763.

Math (reference unrolled, N_STEPS=5, FFI_DELAY=2, FBI_DELAY=20 > N_STEPS so
the FBI masks are dead code):

    drive = amp * clip(ffi_scale,0.01) * 0.5 * (1 + cos(phase))
    md    = mean(drive);  m0 = 0.3*md;  m1 = 0.51*md
    p0    = relu(drive - m0)
    m2    = 0.357*md + 0.3*mean(p0)
    ema5  = 0.17493*drive + 0.147*p0 + 0.21*relu(drive-m1) + 0.3*relu(drive-m2)
    out   = where(ema5 >= kth_largest(ema5, 32), ema5, 0)

Key facts used:
  * ema5 is a strictly increasing per-row function of drive, so the top-32
    mask of ema5 equals the top-32 mask of drive (dd below, unscaled).
  * The top-32 threshold is far above m0/m1/m2 (checked per-row via stats;
    host-fixed otherwise), so on selected elements every relu is affine:
    ema5 = A*dd + B_row with per-row B from two row-sums.
  * Top-32 per row: per-chunk top-8 via the DVE Max8 op, then 4 rounds of
    max+match_replace over the candidates. Exact unless a chunk holds >=9 of
    the row's top-32; detected per row (m8 >= th) and recomputed on host.
  * Selection applied WITHOUT a compare pass: smask = Sign(dd - th_eps) on
    ACT with th_eps = th*(1-2^-20) < th, u = smask*dd on Pool, and
    out = Relu(A*u + B_row) on ACT -- unselected rows have u = -dd <= 0 so
    A*u+B < 0 and Relu zeroes them. Rows where some dd lands in
    (th_eps, th) over-select; detected on host via nonzero counts != 32 and
    recomputed exactly.

Engine layout (all combos verified against the walrus ISA checker):
  ACT : sin, q-relu(+S0 accum), Sign-mask, out-relu(->bf16) + [P,1] tinies
  Pool: e = h*amp, dd = e*h, u = smask*dd          (plain TensorTensor mult)
  DVE : Sdd row-sum (tensor_reduce) + top-k selection
  SP  : one interleaved 1MB load per tile (prefetched 3 ahead), bf16 store,
        single tail stats DMA.

Sharding: pure data parallel, 4096 rows per core on 8 cores.
"""
import sys

for _p in ("/opt/trn_rl_repo", "/root/.axon_site/_ro/trn_rl_repo"):
    if _p not in sys.path:
        sys.path.insert(0, _p)

import numpy as np

import concourse.bass as bass
import concourse.bacc as bacc
import concourse.tile as tile
import concourse.mybir as mybir
from concourse.bass_utils import run_bass_kernel_spmd

AF = mybir.ActivationFunctionType
OP = mybir.AluOpType
F32 = mybir.dt.float32
BF16 = mybir.dt.bfloat16

B_FULL, N = 32768, 1024
NCORES = 8
ROWS = B_FULL // NCORES      # 4096 rows per core
P = 128                      # SBUF partitions
TILES = ROWS // P            # 32 tiles per core
NEG_INF = -3.0e38
HALF_PI = float(np.float32(np.pi / 2))
EPS_SCALE = float(np.float32(1.0 - 2.0 ** -20))

CFG = dict(
    chunk=64,          # selection chunk width (64 -> 16 max8 calls)
    out_dtype="bf16",  # "bf16" or "f32" store tile
    prefetch=3,        # input loads issued this many tiles ahead
    io_bufs=4, out_bufs=3, mid_bufs=3, sel_bufs=2,
    sdd_split=0,       # every k-th tile computes Sdd on ACT instead of DVE
    repeats=1,         # python-unrolled repeats of the whole pipeline
)

_cache = {}


def _build(s: float, cfg: dict | None = None):
    cfg = {**CFG, **(cfg or {})}
    key = (s, tuple(sorted(cfg.items())))
    if key in _cache:
        return _cache[key]

    C = cfg["chunk"]
    G = N // C
    NCAND = G * 8
    ODT = BF16 if cfg["out_dtype"] == "bf16" else F32

    A_imm = float(np.float32(s * 0.83193))
    c_beta0 = float(np.float32(-0.3 / N))
    c_B1 = float(np.float32(-s * 0.25836 / N))
    c_B2 = float(np.float32(-s * 0.09 / N))

    nc = bacc.Bacc("TRN2", target_bir_lowering=False, debug=False)

    _pihalf = nc.alloc_sbuf_tensor("const-pihalf", [P, 1], F32)
    nc.gpsimd.memset(_pihalf.ap(), HALF_PI)
    nc.const_aps.aps[(F32, HALF_PI)] = _pihalf.ap()
    nc.all_engine_barrier()

    pa_d = nc.dram_tensor("pa", [ROWS, 2 * N], F32, kind="ExternalInput")
    out_d = nc.dram_tensor("out", [ROWS, N], ODT, kind="ExternalOutput")
    # per-row stats: [th, m8, Sdd, S0]
    stats_d = nc.dram_tensor("stats", [ROWS, 4], F32, kind="ExternalOutput")

    pa_t = pa_d.ap().rearrange("(t p) n -> t p n", p=P)
    out_t = out_d.ap().rearrange("(t p) n -> t p n", p=P)
    stats_h = stats_d.ap().rearrange("(t p) c -> p t c", p=P)

    PF = cfg["prefetch"]
    with tile.TileContext(nc) as tc:
        with tc.tile_pool(name="io", bufs=cfg["io_bufs"]) as io, \
             tc.tile_pool(name="op", bufs=cfg["out_bufs"]) as op, \
             tc.tile_pool(name="mid", bufs=cfg["mid_bufs"]) as mid, \
             tc.tile_pool(name="sel", bufs=cfg["sel_bufs"]) as selp, \
             tc.tile_pool(name="stp", bufs=1) as stp:
            for rep in range(cfg["repeats"]):
                sa = stp.tile([P, TILES * 4], F32, tag="statsAll")
                inb = [None] * TILES

                def issue_load(t):
                    inb[t] = io.tile([P, 2 * N], F32, tag="in",
                                     name=f"inb{rep}_{t}")
                    nc.sync.dma_start(inb[t][:], pa_t[t])

                for t in range(min(PF, TILES)):
                    issue_load(t)

                for t in range(TILES):
                    phase = inb[t][:, 0:N]
                    amp = inb[t][:, N:2 * N]

                    # h = cos(phase/2)
                    h = mid.tile([P, N], F32, tag="h")
                    nc.scalar.activation(h[:], phase, AF.Sin,
                                         bias=HALF_PI, scale=-0.5)
                    # e = h*amp ; dd = e*h = amp*(1+cos(phase))/2   (Pool)
                    e = mid.tile([P, N], F32, tag="e")
                    nc.gpsimd.tensor_tensor(e[:], h[:], amp, OP.mult)
                    dd = mid.tile([P, N], F32, tag="dd")
                    nc.gpsimd.tensor_tensor(dd[:], e[:], h[:], OP.mult)

                    # Sdd -> sa[:, 4t+2]; balanced between DVE and ACT
                    ks = cfg["sdd_split"]
                    if ks and (t % ks == ks - 1):
                        sdds = mid.tile([P, N], F32, tag="sdds")
                        nc.scalar.activation(
                            sdds[:], dd[:], AF.Copy,
                            accum_out=sa[:, 4 * t + 2:4 * t + 3])
                    else:
                        nc.vector.tensor_reduce(sa[:, 4 * t + 2:4 * t + 3],
                                                dd[:],
                                                mybir.AxisListType.X, OP.add)

                    # beta0 = -m0 = Sdd * (-0.3/N)
                    beta0 = selp.tile([P, 1], F32, tag="beta0")
                    nc.scalar.activation(beta0[:], sa[:, 4 * t + 2:4 * t + 3],
                                         AF.Copy, scale=c_beta0)

                    # S0 = sum(relu(dd - m0)) on ACT (bias = -m0)
                    q = mid.tile([P, N], F32, tag="q")
                    nc.scalar.activation(q[:], dd[:], AF.Relu, bias=beta0[:],
                                         scale=1.0,
                                         accum_out=sa[:, 4 * t + 3:4 * t + 4])

                    # B_row = Sdd*c_B1 + S0*c_B2
                    v2 = selp.tile([P, 1], F32, tag="v2")
                    Bv = selp.tile([P, 1], F32, tag="Bv")
                    nc.scalar.activation(v2[:], sa[:, 4 * t + 3:4 * t + 4],
                                         AF.Copy, scale=c_B2)
                    nc.scalar.activation(Bv[:], sa[:, 4 * t + 2:4 * t + 3],
                                         AF.Identity, bias=v2[:], scale=c_B1)

                    # --- selection on dd (DVE) ---
                    cand = selp.tile([P, NCAND], F32, tag="cand")
                    for j in range(G):
                        nc.vector.max(cand[:, j * 8:(j + 1) * 8],
                                      dd[:, j * C:(j + 1) * C])
                    mrA = selp.tile([P, NCAND], F32, tag="mrA")
                    mrB = selp.tile([P, NCAND], F32, tag="mrB")
                    r1 = selp.tile([P, 8], F32, tag="r1")
                    r2 = selp.tile([P, 8], F32, tag="r2")
                    r3 = selp.tile([P, 8], F32, tag="r3")
                    r4 = selp.tile([P, 8], F32, tag="r4")
                    nc.vector.max(r1[:], cand[:])
                    nc.vector.match_replace(mrA[:], r1[:], cand[:], NEG_INF)
                    nc.vector.max(r2[:], mrA[:])
                    nc.vector.match_replace(mrB[:], r2[:], mrA[:], NEG_INF)
                    nc.vector.max(r3[:], mrB[:])
                    nc.vector.match_replace(mrA[:], r3[:], mrB[:], NEG_INF)
                    nc.vector.max(r4[:], mrA[:])
                    # m8 guard: max over per-chunk 8th-largest
                    nc.vector.tensor_reduce(sa[:, 4 * t + 1:4 * t + 2],
                                            cand[:, 7::8],
                                            mybir.AxisListType.X, OP.max)
                    # th -> stats col 0 ; negated shrunken threshold for Sign
                    nc.scalar.activation(sa[:, 4 * t:4 * t + 1], r4[:, 7:8],
                                         AF.Copy)
                    nth = selp.tile([P, 1], F32, tag="nth")
                    nc.scalar.activation(nth[:], r4[:, 7:8], AF.Copy,
                                         scale=-EPS_SCALE)

                    # smask = sign(dd - th_eps)  in {-1, 0, +1}   (ACT)
                    sm = mid.tile([P, N], F32, tag="sm")
                    nc.scalar.activation(sm[:], dd[:], AF.Sign, bias=nth[:],
                                         scale=1.0)
                    # u = smask * dd   (Pool) : +dd selected / -dd unselected
                    u = mid.tile([P, N], F32, tag="u")
                    nc.gpsimd.tensor_tensor(u[:], sm[:], dd[:], OP.mult)

                    # out = relu(A*u + B_row) -> bf16 (unselected: A*u+B < 0)
                    outb = op.tile([P, N], ODT, tag="out")
                    nc.scalar.activation(outb[:], u[:], AF.Relu, bias=Bv[:],
                                         scale=A_imm)

                    nc.sync.dma_start(out_t[t], outb[:])
                    if t + PF < TILES:
                        issue_load(t + PF)

                nc.sync.dma_start(stats_h, sa[:].rearrange(
                    "p (t c) -> p t c", c=4))

    nc.compile()
    _cache[key] = nc
    return nc


def _interleave(phase, amp):
    """[R, N] + [R, N] -> [R, 2N] with phase in cols 0:N, amp in N:2N."""
    pa = np.empty((phase.shape[0], 2 * N), dtype=np.float32)
    pa[:, 0:N] = phase
    pa[:, N:2 * N] = amp
    return pa


def _reference_rows(phase, amp, s):
    """Exact f32 recompute of the reference for a few rows (host fixup)."""
    f32 = np.float32
    drive = (amp * f32(s) * f32(0.5) *
             (f32(1.0) + np.cos(phase, dtype=f32))).astype(f32)
    ema = np.zeros_like(drive)
    ffi_hist = []
    for t in range(5):
        ffi = ffi_hist[t - 2] if t >= 2 else np.zeros((drive.shape[0], 1), f32)
        inp = np.maximum(drive - ffi, 0)
        ema = (f32(0.7) * ema + f32(0.3) * inp).astype(f32)
        ffi_hist.append(ema.mean(1, keepdims=True, dtype=f32).astype(f32))
    kth = np.sort(ema, 1)[:, ::-1][:, 31:32]
    return np.where(ema >= kth, ema, 0).astype(f32)


def kernel(phase, amplitude, ffi_scale, fbi_temperature):
    phase = np.asarray(phase, dtype=np.float32)
    amplitude = np.asarray(amplitude, dtype=np.float32)
    s = float(np.clip(np.float32(ffi_scale), np.float32(0.01), None))

    nc = _build(s)
    in_maps = [
        {"pa": _interleave(phase[i * ROWS:(i + 1) * ROWS],
                           amplitude[i * ROWS:(i + 1) * ROWS])}
        for i in range(NCORES)
    ]
    res = run_bass_kernel_spmd(nc, in_maps, list(range(NCORES)))
    out = np.concatenate(
        [np.asarray(res.results[i]["out"]).astype(np.float32)
         for i in range(NCORES)], axis=0)
    np.maximum(out, 0.0, out=out)  # clamp residual negatives (unselected)
    stats = np.concatenate([res.results[i]["stats"] for i in range(NCORES)],
                           axis=0)
    th, m8, Sdd, S0 = (stats[:, 0], stats[:, 1], stats[:, 2], stats[:, 3])

    # Host-side validity flags (exactness guards); recompute flagged rows.
    mdd = Sdd / np.float32(N)
    mq0 = S0 / np.float32(N)
    m2 = np.float32(0.357) * mdd + np.float32(0.3) * mq0
    mmax = np.maximum(np.float32(0.51) * mdd, m2)
    nnz = np.count_nonzero(out, axis=1)
    bad = (m8 >= th) | (th <= np.float32(1.05) * mmax) | (nnz != 32)
    import os
    if os.environ.get("DG_DEBUG"):
        print(f"[kernel] flagged rows: {int(bad.sum())}")
    if bad.any():
        idx = np.where(bad)[0]
        out[idx] = _reference_rows(phase[idx], amplitude[idx], s)
    return out


# revision 11
# speedup vs baseline: 1.4022x; 1.2989x over previous
"""Trainium2 Bass kernel for nn_DGLayer_16286515986763.

Math (reference unrolled, N_STEPS=5, FFI_DELAY=2, FBI_DELAY=20 > N_STEPS so
the FBI masks are dead code):

    drive = amp * clip(ffi_scale,0.01) * 0.5 * (1 + cos(phase))
    md    = mean(drive);  m0 = 0.3*md;  m1 = 0.51*md
    p0    = relu(drive - m0)
    m2    = 0.357*md + 0.3*mean(p0)
    ema5  = 0.17493*drive + 0.147*p0 + 0.21*relu(drive-m1) + 0.3*relu(drive-m2)
    out   = where(ema5 >= kth_largest(ema5, 32), ema5, 0)

Key facts used:
  * ema5 is a strictly increasing per-row function of drive, so the top-32
    mask of ema5 equals the top-32 mask of drive (dd below, unscaled).
  * The top-32 threshold is far above m0/m1/m2 (checked per-row via stats;
    host-fixed otherwise), so on selected elements every relu is affine:
    ema5 = A*dd + B_row with per-row B from two row-sums.
  * Top-32 per row: per-chunk top-8 via the DVE Max8 op, then 4 rounds of
    max+match_replace over the candidates. Exact unless a chunk holds >=9 of
    the row's top-32; detected per row (m8 >= th) and recomputed on host.
  * Selection applied WITHOUT a compare pass: smask = Sign(dd - th_eps) on
    ACT with th_eps = th*(1-2^-20) < th, u = smask*dd on Pool, and
    out = Relu(A*u + B_row) on ACT -- unselected rows have u = -dd <= 0 so
    A*u+B < 0 and Relu zeroes them. Rows where some dd lands in
    (th_eps, th) over-select; detected on host via nonzero counts != 32 and
    recomputed exactly.

Engine layout (all combos verified against the walrus ISA checker):
  ACT : sin, q-relu(+S0 accum), Sign-mask, out-relu(->bf16) + [P,1] tinies
  Pool: e = h*amp, dd = e*h, u = smask*dd          (plain TensorTensor mult)
  DVE : Sdd row-sum (tensor_reduce) + top-k selection
  SP  : one interleaved 1MB load per tile (prefetched 3 ahead), bf16 store,
        single tail stats DMA.

Sharding: pure data parallel, 4096 rows per core on 8 cores.
"""
import sys

for _p in ("/opt/trn_rl_repo", "/root/.axon_site/_ro/trn_rl_repo"):
    if _p not in sys.path:
        sys.path.insert(0, _p)

import numpy as np

import concourse.bass as bass
import concourse.bacc as bacc
import concourse.tile as tile
import concourse.mybir as mybir
from concourse.bass_utils import run_bass_kernel_spmd

AF = mybir.ActivationFunctionType
OP = mybir.AluOpType
F32 = mybir.dt.float32
BF16 = mybir.dt.bfloat16

B_FULL, N = 32768, 1024
NCORES = 8
ROWS = B_FULL // NCORES      # 4096 rows per core
P = 128                      # SBUF partitions
TILES = ROWS // P            # 32 tiles per core
NEG_INF = -3.0e38
HALF_PI = float(np.float32(np.pi / 2))
EPS_SCALE = float(np.float32(1.0 - 2.0 ** -20))

CFG = dict(
    chunk=64,          # selection chunk width (64 -> 16 max8 calls)
    out_dtype="bf16",  # "bf16" or "f32" store tile
    group=4,           # tiles per DMA (4 -> 8 loads of 4MB, 8 stores of 1MB)
    io_bufs=2, out_bufs=2, mid_bufs=3, sel_bufs=2,
    sdd_split=0,       # every k-th tile computes Sdd on ACT instead of DVE
    repeats=1,         # python-unrolled repeats of the whole pipeline
)

_cache = {}


def _build(s: float, cfg: dict | None = None):
    cfg = {**CFG, **(cfg or {})}
    key = (s, tuple(sorted(cfg.items())))
    if key in _cache:
        return _cache[key]

    C = cfg["chunk"]
    G = N // C
    NCAND = G * 8
    ODT = BF16 if cfg["out_dtype"] == "bf16" else F32

    A_imm = float(np.float32(s * 0.83193))
    c_beta0 = float(np.float32(-0.3 / N))
    c_B1 = float(np.float32(-s * 0.25836 / N))
    c_B2 = float(np.float32(-s * 0.09 / N))

    nc = bacc.Bacc("TRN2", target_bir_lowering=False, debug=False)

    _pihalf = nc.alloc_sbuf_tensor("const-pihalf", [P, 1], F32)
    nc.gpsimd.memset(_pihalf.ap(), HALF_PI)
    nc.const_aps.aps[(F32, HALF_PI)] = _pihalf.ap()
    nc.all_engine_barrier()

    pa_d = nc.dram_tensor("pa", [ROWS, 2 * N], F32, kind="ExternalInput")
    out_d = nc.dram_tensor("out", [ROWS, N], ODT, kind="ExternalOutput")
    # per-row stats: [th, m8, Sdd, S0]
    stats_d = nc.dram_tensor("stats", [ROWS, 4], F32, kind="ExternalOutput")

    TG = cfg["group"]                  # tiles per DMA group
    NG = TILES // TG
    pa_g = pa_d.ap().rearrange("(g q p) n -> g p q n", p=P, q=TG)
    out_g = out_d.ap().rearrange("(g q p) n -> g p q n", p=P, q=TG)
    stats_h = stats_d.ap().rearrange("(t p) c -> p t c", p=P)

    with tile.TileContext(nc) as tc:
        with tc.tile_pool(name="io", bufs=cfg["io_bufs"]) as io, \
             tc.tile_pool(name="op", bufs=cfg["out_bufs"]) as op, \
             tc.tile_pool(name="mid", bufs=cfg["mid_bufs"]) as mid, \
             tc.tile_pool(name="sel", bufs=cfg["sel_bufs"]) as selp, \
             tc.tile_pool(name="stp", bufs=1) as stp:
            for rep in range(cfg["repeats"]):
                sa = stp.tile([P, TILES * 4], F32, tag="statsAll")
                inb = [None] * NG

                def issue_load(g):
                    inb[g] = io.tile([P, TG * 2 * N], F32, tag="in",
                                     name=f"inb{rep}_{g}")
                    nc.sync.dma_start(
                        inb[g][:].rearrange("p (q n) -> p q n", q=TG),
                        pa_g[g])

                issue_load(0)
                if NG > 1:
                    issue_load(1)

                for t in range(TILES):
                    g, qi = divmod(t, TG)
                    if qi == 0:
                        ob = op.tile([P, TG * N], ODT, tag="out",
                                     name=f"ob{rep}_{g}")
                    phase = inb[g][:, qi * 2 * N:qi * 2 * N + N]
                    amp = inb[g][:, qi * 2 * N + N:(qi + 1) * 2 * N]

                    # h = cos(phase/2)
                    h = mid.tile([P, N], F32, tag="h")
                    nc.scalar.activation(h[:], phase, AF.Sin,
                                         bias=HALF_PI, scale=-0.5)
                    # e = h*amp ; dd = e*h = amp*(1+cos(phase))/2   (Pool)
                    e = mid.tile([P, N], F32, tag="e")
                    nc.gpsimd.tensor_tensor(e[:], h[:], amp, OP.mult)
                    dd = mid.tile([P, N], F32, tag="dd")
                    nc.gpsimd.tensor_tensor(dd[:], e[:], h[:], OP.mult)

                    # Sdd -> sa[:, 4t+2]; balanced between DVE and ACT
                    ks = cfg["sdd_split"]
                    if ks and (t % ks == ks - 1):
                        sdds = mid.tile([P, N], F32, tag="sdds")
                        nc.scalar.activation(
                            sdds[:], dd[:], AF.Copy,
                            accum_out=sa[:, 4 * t + 2:4 * t + 3])
                    else:
                        nc.vector.tensor_reduce(sa[:, 4 * t + 2:4 * t + 3],
                                                dd[:],
                                                mybir.AxisListType.X, OP.add)

                    # beta0 = -m0 = Sdd * (-0.3/N)
                    beta0 = selp.tile([P, 1], F32, tag="beta0")
                    nc.scalar.activation(beta0[:], sa[:, 4 * t + 2:4 * t + 3],
                                         AF.Copy, scale=c_beta0)

                    # S0 = sum(relu(dd - m0)) on ACT (bias = -m0)
                    q = mid.tile([P, N], F32, tag="q")
                    nc.scalar.activation(q[:], dd[:], AF.Relu, bias=beta0[:],
                                         scale=1.0,
                                         accum_out=sa[:, 4 * t + 3:4 * t + 4])

                    # B_row = Sdd*c_B1 + S0*c_B2
                    v2 = selp.tile([P, 1], F32, tag="v2")
                    Bv = selp.tile([P, 1], F32, tag="Bv")
                    nc.scalar.activation(v2[:], sa[:, 4 * t + 3:4 * t + 4],
                                         AF.Copy, scale=c_B2)
                    nc.scalar.activation(Bv[:], sa[:, 4 * t + 2:4 * t + 3],
                                         AF.Identity, bias=v2[:], scale=c_B1)

                    # --- selection on dd (DVE) ---
                    cand = selp.tile([P, NCAND], F32, tag="cand")
                    for j in range(G):
                        nc.vector.max(cand[:, j * 8:(j + 1) * 8],
                                      dd[:, j * C:(j + 1) * C])
                    mrA = selp.tile([P, NCAND], F32, tag="mrA")
                    mrB = selp.tile([P, NCAND], F32, tag="mrB")
                    r1 = selp.tile([P, 8], F32, tag="r1")
                    r2 = selp.tile([P, 8], F32, tag="r2")
                    r3 = selp.tile([P, 8], F32, tag="r3")
                    r4 = selp.tile([P, 8], F32, tag="r4")
                    nc.vector.max(r1[:], cand[:])
                    nc.vector.match_replace(mrA[:], r1[:], cand[:], NEG_INF)
                    nc.vector.max(r2[:], mrA[:])
                    nc.vector.match_replace(mrB[:], r2[:], mrA[:], NEG_INF)
                    nc.vector.max(r3[:], mrB[:])
                    nc.vector.match_replace(mrA[:], r3[:], mrB[:], NEG_INF)
                    nc.vector.max(r4[:], mrA[:])
                    # m8 guard: max over per-chunk 8th-largest
                    nc.vector.tensor_reduce(sa[:, 4 * t + 1:4 * t + 2],
                                            cand[:, 7::8],
                                            mybir.AxisListType.X, OP.max)
                    # th -> stats col 0 ; negated shrunken threshold for Sign
                    nc.scalar.activation(sa[:, 4 * t:4 * t + 1], r4[:, 7:8],
                                         AF.Copy)
                    nth = selp.tile([P, 1], F32, tag="nth")
                    nc.scalar.activation(nth[:], r4[:, 7:8], AF.Copy,
                                         scale=-EPS_SCALE)

                    # smask = sign(dd - th_eps)  in {-1, 0, +1}   (ACT)
                    sm = mid.tile([P, N], F32, tag="sm")
                    nc.scalar.activation(sm[:], dd[:], AF.Sign, bias=nth[:],
                                         scale=1.0)
                    # u = smask * dd   (Pool) : +dd selected / -dd unselected
                    u = mid.tile([P, N], F32, tag="u")
                    nc.gpsimd.tensor_tensor(u[:], sm[:], dd[:], OP.mult)

                    # out = relu(A*u + B_row) -> bf16 (unselected: A*u+B < 0)
                    nc.scalar.activation(ob[:, qi * N:(qi + 1) * N], u[:],
                                         AF.Relu, bias=Bv[:], scale=A_imm)

                    if qi == TG - 1:
                        # store the finished group on the ACT HWDGE ring so
                        # loads (SP ring) never queue behind stores
                        nc.scalar.dma_start(
                            out_g[g],
                            ob[:].rearrange("p (q n) -> p q n", q=TG))
                        if g + 2 < NG:
                            issue_load(g + 2)

                nc.sync.dma_start(stats_h, sa[:].rearrange(
                    "p (t c) -> p t c", c=4))

    nc.compile()
    _cache[key] = nc
    return nc


def _interleave(phase, amp):
    """[R, N] + [R, N] -> [R, 2N] with phase in cols 0:N, amp in N:2N."""
    pa = np.empty((phase.shape[0], 2 * N), dtype=np.float32)
    pa[:, 0:N] = phase
    pa[:, N:2 * N] = amp
    return pa


def _reference_rows(phase, amp, s):
    """Exact f32 recompute of the reference for a few rows (host fixup)."""
    f32 = np.float32
    drive = (amp * f32(s) * f32(0.5) *
             (f32(1.0) + np.cos(phase, dtype=f32))).astype(f32)
    ema = np.zeros_like(drive)
    ffi_hist = []
    for t in range(5):
        ffi = ffi_hist[t - 2] if t >= 2 else np.zeros((drive.shape[0], 1), f32)
        inp = np.maximum(drive - ffi, 0)
        ema = (f32(0.7) * ema + f32(0.3) * inp).astype(f32)
        ffi_hist.append(ema.mean(1, keepdims=True, dtype=f32).astype(f32))
    kth = np.sort(ema, 1)[:, ::-1][:, 31:32]
    return np.where(ema >= kth, ema, 0).astype(f32)


def kernel(phase, amplitude, ffi_scale, fbi_temperature):
    phase = np.asarray(phase, dtype=np.float32)
    amplitude = np.asarray(amplitude, dtype=np.float32)
    s = float(np.clip(np.float32(ffi_scale), np.float32(0.01), None))

    nc = _build(s)
    in_maps = [
        {"pa": _interleave(phase[i * ROWS:(i + 1) * ROWS],
                           amplitude[i * ROWS:(i + 1) * ROWS])}
        for i in range(NCORES)
    ]
    res = run_bass_kernel_spmd(nc, in_maps, list(range(NCORES)))
    out = np.concatenate(
        [np.asarray(res.results[i]["out"]).astype(np.float32)
         for i in range(NCORES)], axis=0)
    np.maximum(out, 0.0, out=out)  # clamp residual negatives (unselected)
    stats = np.concatenate([res.results[i]["stats"] for i in range(NCORES)],
                           axis=0)
    th, m8, Sdd, S0 = (stats[:, 0], stats[:, 1], stats[:, 2], stats[:, 3])

    # Host-side validity flags (exactness guards); recompute flagged rows.
    mdd = Sdd / np.float32(N)
    mq0 = S0 / np.float32(N)
    m2 = np.float32(0.357) * mdd + np.float32(0.3) * mq0
    mmax = np.maximum(np.float32(0.51) * mdd, m2)
    nnz = np.count_nonzero(out, axis=1)
    bad = (m8 >= th) | (th <= np.float32(1.05) * mmax) | (nnz != 32)
    import os
    if os.environ.get("DG_DEBUG"):
        print(f"[kernel] flagged rows: {int(bad.sum())}")
    if bad.any():
        idx = np.where(bad)[0]
        out[idx] = _reference_rows(phase[idx], amplitude[idx], s)
    return out
